# revision 4
# baseline (speedup 1.0000x reference)
"""Trainium2 Bass kernel for XCiT-style channel ("cross-covariance") attention.

Reference computation (per batch element b):
    qkv  = x @ w_qkv.T                    # [N, 3C]
    q,k,v -> [H, DH, N] (channel-major)
    q,k  l2-normalized along N (tokens)
    attn = softmax((q @ k^T) * temp)      # [H, DH, DH]
    out  = (attn @ v) -> [N, C] @ w_proj.T

Shapes: B=8, N=4096, C=512, H=8, DH=64.

Strategy: data-parallel over batch across the 8 NeuronCores (one batch
element per core, weights replicated, no collectives).

Key algebraic restructure v2: neither q, k nor v are ever materialized.
Everything the attention needs is a function of the token Gram matrix
    Xg = x^T x                          # [C, C], per batch element
since
    q_h^T k_h   = wq_h Xg wk_h^T        # per-head [DH, DH] logits
    ||q_d||^2   = (wq Xg wq^T)[d, d]
    ||k_e||^2   = (wk Xg wk^T)[e, e]
and (folding v and the projection, as before)
    weff[he, co] = sum_d attn_h[d, e] * w_proj^T[hd, co]
    G = w_v^T @ weff                    # [cin, cout]
    out = x @ G
This removes the [N, 2C] q/k intermediate entirely: the only O(N)
matmuls left are Xg (fp8 DoubleRow, 2x PE rate) and x @ G (bf16).

Pipeline per core:
    Xg   = x8^T x8          fp8 DR, accumulate over token-tile pairs
    Z    = Xg @ [wq^T|wk^T] bf16    ([cin, 2C]; Xg symmetric -> Z = [wq Xg | wk Xg]^T)
    QS_p = Zq_p^T. gram: [Q2_p | S_p] = Z_p^T over cin vs [wq_p | wk_p]
    dk   = ones^T (Zk o wk^T)       row of ||k||^2 via ones-matmul
    softmax (rq = temp*rsqrt(diag Q2), rk = rsqrt(dk)) -> attn block-diag
    weff = attn @ wpT;  G = wv^T weff;  out = x @ G
"""

import numpy as np

import concourse.bacc as bacc
import concourse.mybir as mybir
import concourse.tile as tile

F32 = mybir.dt.float32
BF16 = mybir.dt.bfloat16
F8 = mybir.dt.float8e4
DR = mybir.MatmulPerfMode.DoubleRow

N_TOK = 4096
C = 512
H = 8
DH = 64
P = 128
KT = C // P            # 4 cin tiles
NT = N_TOK // P        # 32 token tiles
NTP = NT // 2          # 16 token tile pairs
NCH = N_TOK // 512     # 8 output chunks
TPC = 4                # token tiles per chunk
N_CORES = 8

CFG = {"phases": "WABC", "psy_bufs": 3, "hint": True, "xg_dr": True}

_COMBINED_TABLE = "natural_log_exp_and_others"


class _Bacc(bacc.Bacc):
    """Bacc with a single combined ACT function table.

    The stock inserter picks the first table containing each activation
    function (Ln -> natural_log, Exp/Copy -> exp_and_others), so a
    Copy/Ln/Exp mix thrashes 1.28us table loads right on the softmax
    critical path. Every function this kernel uses (copy, ln, exp) lives
    in natural_log_exp_and_others, so retarget all loads there and drop
    the now-redundant ones (they carry no sync info).
    """

    def insert_act_table_loads(self):
        super().insert_act_table_loads()
        from concourse.hw_specs import get_activation_tables

        tables = get_activation_tables(self.m.arch)
        names = list(tables)
        combined_id = names.index(_COMBINED_TABLE)
        allowed = tables[_COMBINED_TABLE]
        for b in self.main_func.blocks:
            first = True
            keep = []
            for inst in b.instructions:
                if isinstance(inst, mybir.InstActivation):
                    assert inst.func in allowed, inst.func
                if isinstance(inst, mybir.InstLoadActFuncSet):
                    si = inst.sync_info
                    assert si is None or (not si.on_wait and not si.on_update)
                    if first:
                        inst.act_func_set_id = combined_id
                        first = False
                    else:
                        continue
                keep.append(inst)
            b.instructions[:] = keep


def build_bass(loop_n=None):
    nc = _Bacc() if CFG.get("act_fix", True) else bacc.Bacc()

    if CFG.get("xg_mode", "dr") == "drsw":
        # column-interleaved+reversed weight layout (DoubleRowSwInterleave)
        x8_d = nc.declare_dram_parameter("x8", [P, NT * C], F8, isOutput=False)
    else:
        x8_d = nc.declare_dram_parameter("x8", [N_TOK, C], F8, isOutput=False)
    xTb_d = nc.declare_dram_parameter("xTb", [C, N_TOK], BF16, isOutput=False)
    wqkT_d = nc.declare_dram_parameter("wqkT", [C, 2 * C], BF16, isOutput=False)
    wqk8_d = nc.declare_dram_parameter("wqk8", [C, 2 * C], F8, isOutput=False)
    wv_d = nc.declare_dram_parameter("wv", [C, C], BF16, isOutput=False)
    wpT_d = nc.declare_dram_parameter("wpT", [C, C], BF16, isOutput=False)
    temp_d = nc.declare_dram_parameter("temperature", [H, 1, 1], F32, isOutput=False)
    id_d = nc.declare_dram_parameter("ident", [P, P], F32, isOutput=False)
    if CFG.get("c_transposed", False):
        out_d = nc.declare_dram_parameter("out", [C, N_TOK], BF16, isOutput=True)
    else:
        out_d = nc.declare_dram_parameter("out", [N_TOK, C], BF16, isOutput=True)

    with tile.TileContext(nc) as tc:
        with tc.tile_pool(name="persist", bufs=1) as persist:
            onesb = persist.tile([P, 1], BF16, tag="onesb")
            nc.gpsimd.memset(onesb[:], 1.0)
            ones2 = persist.tile([P, P], BF16, tag="ones2")
            nc.gpsimd.memset(ones2[:], 1.0)
            # additive block-diagonal mask: 0 on the per-head diagonal
            # blocks, -30 off them (exp -> ~1e-13, vanishes in the softmax)
            mask_bd = persist.tile([P, P], F32, tag="mask_bd")
            nc.gpsimd.memset(mask_bd[:], -3000.0)
            nc.gpsimd.memset(mask_bd[0:DH, 0:DH], 0.0)
            nc.gpsimd.memset(mask_bd[DH:P, DH:P], 0.0)
            idm = persist.tile([P, P], F32, tag="idm")
            scrp = persist.tile([P, P], F32, tag="scrp")
            t8 = persist.tile([1, H], F32, tag="t8")
            tcol = persist.tile([P, KT], F32, tag="tcol")

            # persistent SBUF tensors
            if CFG.get("xg_mode", "dr") == "drsw":
                x8 = persist.tile([P, NTP, KT, 2 * P], F8, tag="x8")
            else:
                x8 = persist.tile([P, NT, C], F8, tag="x8")
            xTb = persist.tile([P, KT, N_TOK], BF16, tag="xTb")
            wqkT = persist.tile([P, KT, 2 * C], BF16, tag="wqkT")
            wv_sb = persist.tile([P, KT, C], BF16, tag="wv_sb")
            wpT_sb = persist.tile([P, KT, C], BF16, tag="wpT_sb")
            xg_sb = persist.tile([P, KT, C], F8, tag="xg_sb")
            wqk8 = persist.tile([P, KT, 2 * C], F8, tag="wqk8")
            z_sb = persist.tile([P, KT, 2 * C], BF16, tag="z_sb")
            pk_sb = persist.tile([P, KT, C], BF16, tag="pk_sb")
            weff = persist.tile([P, KT, C], BF16, tag="weff")
            g_sb = persist.tile([P, KT, C], BF16, tag="g_sb")
            d2 = persist.tile([P, KT], F32, tag="d2")
            rq_col = persist.tile([P, KT], F32, tag="rq_col")
            rk_row = persist.tile([1, C], F32, tag="rk_row")
            rk_bcast = persist.tile([P, C], F32, tag="rk_bcast")

            locals_d = dict(
                onesb=onesb, ones2=ones2, mask_bd=mask_bd, idm=idm,
                scrp=scrp, t8=t8,
                tcol=tcol, temp_d=temp_d, id_d=id_d, x8=x8, xTb=xTb,
                wqkT=wqkT, wqk8=wqk8, wv_sb=wv_sb, wpT_sb=wpT_sb,
                xg_sb=xg_sb,
                z_sb=z_sb, pk_sb=pk_sb, weff=weff, g_sb=g_sb, d2=d2,
                rq_col=rq_col, rk_row=rk_row, rk_bcast=rk_bcast,
                x8_d=x8_d, xTb_d=xTb_d, wqkT_d=wqkT_d, wqk8_d=wqk8_d,
                wv_d=wv_d,
                wpT_d=wpT_d, out_d=out_d,
            )

            def load_consts():
                # ACT-ring DGE: runs concurrently with the x8 stream on SP
                nc.scalar.dma_start(
                    out=wqkT[:], in_=wqkT_d.rearrange("(k p) c -> p k c", p=P))
                nc.scalar.dma_start(
                    out=wqk8[:], in_=wqk8_d.rearrange("(k p) c -> p k c", p=P))
                nc.scalar.dma_start(
                    out=wpT_sb[:], in_=wpT_d.rearrange("(k p) c -> p k c", p=P))
                nc.scalar.dma_start(
                    out=wv_sb[:], in_=wv_d.rearrange("(k p) c -> p k c", p=P))
                nc.scalar.dma_start(out=idm[:], in_=id_d[:])
                # t8[0, two*4 + j] = temperature[2j + two]  (two-major)
                for two in range(2):
                    nc.sync.dma_start(
                        out=t8[0:1, two * KT:(two + 1) * KT],
                        in_=temp_d.rearrange(
                            "(j two) a b -> (a b) two j", two=2)[:, two, :],
                    )
                # tcol[p, j] = temperature[2j + p//64]
                for two in range(2):
                    nc.sync.dma_start(
                        out=tcol[two * DH:(two + 1) * DH, :],
                        in_=t8[0:1, two * KT:(two + 1) * KT]
                        .unsqueeze(1).broadcast_to((1, DH, KT)),
                    )

            def phases():
                _emit(nc, tc, locals_d)

            if loop_n is None:
                load_consts()
                phases()
            else:
                load_consts()
                hint = tuple(nc.engines.keys()) if CFG.get("hint") else ()
                with tc.For_i(0, loop_n, 1, hint_engines=hint):
                    phases()

    nc.compile()
    return nc


def _emit(nc, tc, L):
    onesb, ones2, idm, tcol = L["onesb"], L["ones2"], L["idm"], L["tcol"]
    mask_bd, scrp = L["mask_bd"], L["scrp"]
    t8, temp_d, id_d = L["t8"], L["temp_d"], L["id_d"]
    x8, xTb, wqkT, wqk8 = L["x8"], L["xTb"], L["wqkT"], L["wqk8"]
    wv_sb, wpT_sb = L["wv_sb"], L["wpT_sb"]
    xg_sb, z_sb, pk_sb = L["xg_sb"], L["z_sb"], L["pk_sb"]
    weff, g_sb, d2 = L["weff"], L["g_sb"], L["d2"]
    rq_col, rk_row, rk_bcast = L["rq_col"], L["rk_row"], L["rk_bcast"]
    x8_d, xTb_d, wqkT_d = L["x8_d"], L["xTb_d"], L["wqkT_d"]
    wv_d, wpT_d, out_d = L["wv_d"], L["wpT_d"], L["out_d"]
    phases_on = CFG.get("phases", "WABC")

    copy_flip = [0]

    def copy_out(dst_ap, src_ap):
        """PSUM->SBUF evacuation, alternating DVE / ACT."""
        if copy_flip[0] % 2 == 0:
            nc.vector.tensor_copy(out=dst_ap, in_=src_ap)
        else:
            nc.scalar.copy(out=dst_ap, in_=src_ap)
        copy_flip[0] += 1

    if "W" in phases_on:
        # token-major fp8 x first so Xg can start on the first tile pairs
        if CFG.get("xg_mode", "dr") == "drsw":
            x8_src = x8_d.rearrange("p (tp r) -> p tp r", tp=NTP)
            x8v = x8[:].rearrange("p tp i r -> p tp (i r)")
            for lo, hi in ((0, 2), (2, 8), (8, 16)):
                nc.sync.dma_start(out=x8v[:, lo:hi, :], in_=x8_src[:, lo:hi, :])
        else:
            x8_src = x8_d.rearrange("(t p) c -> p t c", p=P)
            for lo, hi in ((0, 4), (4, 16), (16, 32)):
                nc.sync.dma_start(out=x8[:, lo:hi, :], in_=x8_src[:, lo:hi, :])
        if CFG.get("xtb_split", False):
            for k in range(KT):
                nc.sync.dma_start(out=xTb[:, k, :],
                                  in_=xTb_d[k * P:(k + 1) * P, :])
        else:
            nc.sync.dma_start(
                out=xTb[:], in_=xTb_d.rearrange("(k p) n -> p k n", p=P))

    if "A" in phases_on:
        # --- Xg = x^T x (fp8, DoubleRow over token-tile pairs) ---
        psxg = tc.alloc_tile_pool(name="psxg", bufs=1, space="PSUM")
        xg_ps = [psxg.tile([P, C], F32, tag=f"xg{i}", name=f"xg{i}")
                 for i in range(KT)]
        xg_mode = CFG.get("xg_mode", "dr")
        if xg_mode == "drsw":
            DRS = mybir.MatmulPerfMode.DoubleRowSwInterleave
            for tp in range(NTP):
                # rhs[k, ab, (i, m)] = x8i[k, tp, i, 2*(127-m)+ab]
                rhs = x8[:, tp, :, :].rearrange(
                    "p i (mrev two) -> p two i mrev", two=2)[:, :, :, ::-1]
                for i in range(KT):
                    nc.tensor.matmul(
                        xg_ps[i][:].rearrange("p (i m) -> p i m", m=P),
                        x8[:, tp, i, :],
                        rhs,
                        start=(tp == 0), stop=(tp == NTP - 1),
                        perf_mode=DRS,
                    )
        elif CFG.get("xg_dr", True):
            for tp in range(NTP):
                for i in range(KT):
                    nc.tensor.matmul(
                        xg_ps[i][:],
                        x8[:, 2 * tp:2 * tp + 2, i * P:(i + 1) * P],
                        x8[:, 2 * tp:2 * tp + 2, :],
                        start=(tp == 0), stop=(tp == NTP - 1),
                        perf_mode=DR,
                    )
        else:
            for t in range(NT):
                for i in range(KT):
                    nc.tensor.matmul(
                        xg_ps[i][:],
                        x8[:, t, i * P:(i + 1) * P],
                        x8[:, t, :],
                        start=(t == 0), stop=(t == NT - 1),
                    )
        # half-width evacuations on both engines halve the Xg->Z latency;
        # 1/64-scaled fp8 (the softmax normalization cancels the scale)
        for i in range(KT):
            nc.vector.tensor_scalar_mul(xg_sb[:, i, 0:C // 2],
                                        xg_ps[i][:, 0:C // 2], 1.0 / 64.0)
            nc.scalar.activation(xg_sb[:, i, C // 2:C],
                                 xg_ps[i][:, C // 2:C],
                                 mybir.ActivationFunctionType.Copy,
                                 bias=0.0, scale=1.0 / 64.0)
        psxg.release()

    # fused per-pair-block grams [Q2_p | S_p] (two p-blocks per PSUM bank)
    # + dk row; persists into B
    psq = tc.alloc_tile_pool(name="psq", bufs=1, space="PSUM")
    qs_t = [psq.tile([P, 2, 2, P], F32, tag=f"qs{pp}", name=f"qs{pp}")
            for pp in range(2)]
    # dk broadcast to all partitions directly (all-ones stationary operand)
    dk_ps = psq.tile([P, C], F32, tag="dkps")

    def qs(p):
        return qs_t[p // 2][:, p % 2, :, :]

    if "A" in phases_on:
        # --- Z = Xg @ [wq^T | wk^T]  (bf16), in [cin, qch|kch] layout ---
        # The ||k||^2 ones-matmuls interleave into the Z stream so the
        # rk chain (ln/exp) runs under the QS grams.
        psz = tc.alloc_tile_pool(name="psz", bufs=2, space="PSUM")
        for j in range(KT):
            z_ps = psz.tile([P, 2 * C], F32, tag="z", name=f"z{j}")
            for ii in range(KT // 2):
                for half in range(2):
                    nc.tensor.matmul(
                        z_ps[:, half * C:(half + 1) * C],
                        xg_sb[:, 2 * ii:2 * ii + 2, j * P:(j + 1) * P],
                        wqk8[:, 2 * ii:2 * ii + 2, half * C:(half + 1) * C],
                        start=(ii == 0), stop=(ii == KT // 2 - 1),
                        perf_mode=DR,
                    )
            nc.vector.tensor_copy(out=z_sb[:, j, 0:C], in_=z_ps[:, 0:C])
            nc.scalar.copy(out=z_sb[:, j, C:2 * C], in_=z_ps[:, C:2 * C])
            # Zk o wk^T product feeding the ||k||^2 ones-matmul
            nc.vector.tensor_mul(
                out=pk_sb[:, j, :], in0=z_sb[:, j, C:2 * C],
                in1=wqkT[:, j, C:2 * C],
            )
            if j >= 1:
                jt = j - 1
                nc.tensor.matmul(dk_ps[:], ones2[:], pk_sb[:, jt, :],
                                 start=(jt == 0), stop=False)
        nc.tensor.matmul(dk_ps[:], ones2[:], pk_sb[:, KT - 1, :],
                         start=False, stop=True)
        psz.release()

    def emit_gram(p):
        if "A" not in phases_on:
            return
        # fused [Q2_p | S_p] gram (two p per bank, group-sequential) with
        # the diag(Q2) extraction on DVE inside the stream
        for jt in range(KT):
            rhs = wqkT[:, jt, :].rearrange("p (two c) -> p two c", two=2)[
                :, :, p * P:(p + 1) * P]
            nc.tensor.matmul(
                qs(p), z_sb[:, jt, p * P:(p + 1) * P], rhs,
                start=(jt == 0), stop=(jt == KT - 1),
            )
        nc.vector.tensor_mul(out=scrp[:], in0=qs(p)[:, 0, :], in1=idm[:])
        nc.vector.reduce_sum(d2[:, p:p + 1], scrp[:],
                             axis=mybir.AxisListType.X)

    if "A" in phases_on and "B" not in phases_on:
        for p in range(KT):
            emit_gram(p)

    if "B" in phases_on:
        with (
            tc.tile_pool(name="smp", bufs=2) as smp,
            tc.tile_pool(name="psw", bufs=2, space="PSUM") as psw,
        ):
            # rk_bcast = rsqrt(dk) on the already-broadcast [P, C] block
            # (ACT runs under the QS grams; no PE/DVE step needed)
            lnk = smp.tile([P, C], F32, tag="lnk")
            nc.scalar.activation(lnk[:], dk_ps[:], mybir.ActivationFunctionType.Ln)
            nc.scalar.activation(rk_bcast[:], lnk[:],
                                 mybir.ActivationFunctionType.Exp,
                                 bias=0.0, scale=-0.5)

            psg2 = tc.alloc_tile_pool(name="psg2", bufs=1, space="PSUM")
            gps = [psg2.tile([P, C], F32, tag=f"gp{j}", name=f"gp{j}")
                   for j in range(2)]

            def softmax_p(p):
                # rq_p = temp * rsqrt(d2[:, p]) (tiny per-p chain so the
                # softmax for p runs while later grams still stream)
                lnq = smp.tile([P, 1], F32, tag="lnq")
                nc.scalar.activation(lnq[:], d2[:, p:p + 1],
                                     mybir.ActivationFunctionType.Ln)
                rsq = smp.tile([P, 1], F32, tag="rsq")
                nc.scalar.activation(rsq[:], lnq[:],
                                     mybir.ActivationFunctionType.Exp,
                                     bias=0.0, scale=-0.5)
                nc.vector.tensor_mul(out=rq_col[:, p:p + 1], in0=rsq[:],
                                     in1=tcol[:, p:p + 1])
                # smt = S * rk, + additive block-diag mask; the rq factor
                # rides the exp's per-partition scale (mask is -3000 so it
                # still kills the off-head quadrants after scaling)
                smt = smp.tile([P, P], F32, tag="smt")
                nc.vector.tensor_mul(
                    out=smt[:], in0=qs(p)[:, 1, :],
                    in1=rk_bcast[:, p * P:(p + 1) * P],
                )
                smtm = smp.tile([P, P], F32, tag="smtm")
                nc.vector.tensor_add(out=smtm[:], in0=smt[:], in1=mask_bd[:])
                et = smp.tile([P, P], F32, tag="et")
                ssum = smp.tile([P, 1], F32, tag="ssum")
                rs = smp.tile([P, 1], F32, tag="rs")
                nc.scalar.activation(et[:], smtm[:],
                                     mybir.ActivationFunctionType.Exp,
                                     bias=0.0, scale=rq_col[:, p:p + 1],
                                     accum_out=ssum[:, 0:1])
                nc.vector.reciprocal(rs[:], ssum[:])
                abd = smp.tile([P, P], BF16, tag="abd")
                nc.vector.tensor_scalar_mul(abd[:], et[:], rs[:, 0:1])
                return abd

            def weff_g(p, abd):
                wps = psw.tile([P, C], F32, tag="wps")
                nc.tensor.matmul(wps[:], abd[:], wpT_sb[:, p, :],
                                 start=True, stop=True)
                copy_out(weff[:, p, :], wps[:])
                for j in range(2):
                    nc.tensor.matmul(
                        gps[j][:], wv_sb[:, p, j * P:(j + 1) * P],
                        weff[:, p, :],
                        start=(p == 0), stop=(p == KT - 1),
                    )

            abds = []
            for p in range(KT):
                emit_gram(p)
                if p >= 1:
                    weff_g(p - 1, abds[p - 1])
                abds.append(softmax_p(p))
            weff_g(KT - 1, abds[KT - 1])
            # G blocks 2,3 after the last weff
            for j in range(2):
                copy_out(g_sb[:, j, :], gps[j][:])
            for j in range(2):
                gp = psg2.tile([P, C], F32, tag=f"gp{j}", name=f"gp{j + 2}")
                for t in range(KT):
                    nc.tensor.matmul(
                        gp[:], wv_sb[:, t, (j + 2) * P:(j + 3) * P],
                        weff[:, t, :],
                        start=(t == 0), stop=(t == KT - 1),
                    )
                copy_out(g_sb[:, j + 2, :], gp[:])
            psg2.release()

    stub = []
    if "A" not in phases_on:
        stub = [xg_sb, z_sb, pk_sb, weff, g_sb, rq_col, rk_bcast, d2, rk_row]
    elif "B" not in phases_on:
        stub = [weff, g_sb]
    for t_ in stub:
        nc.gpsimd.memset(t_[:], 0.0)

    psq.release()

    if "C" in phases_on and CFG.get("c_transposed", False):
        # out^T = G^T x : stationary = G blocks, moving = xTb token chunks
        # (FD=1024) -> half the matmul/LDW count of the direct form
        with (
            tc.tile_pool(name="yp", bufs=3) as yp,
            tc.tile_pool(name="psy", bufs=3, space="PSUM") as psy,
        ):
            NCK = N_TOK // 1024
            for ck in range(NCK):
                yc = yp.tile([P, KT, 1024], BF16, tag="yct")
                for co in range(KT):
                    ps = psy.tile([P, 1024], F32, tag="psyt")
                    for k in range(KT):
                        nc.tensor.matmul(
                            ps[:], g_sb[:, k, co * P:(co + 1) * P],
                            xTb[:, k, ck * 1024:(ck + 1) * 1024],
                            start=(k == 0), stop=(k == KT - 1),
                        )
                    copy_out(yc[:, co, :], ps[:])
                nc.sync.dma_start(
                    out=out_d[:, ck * 1024:(ck + 1) * 1024].rearrange(
                        "(co p) n -> p co n", p=P),
                    in_=yc[:],
                )
    elif "C" in phases_on:
        with (
            tc.tile_pool(name="yp", bufs=3) as yp,
            tc.tile_pool(name="psy", bufs=CFG["psy_bufs"], space="PSUM") as psy,
        ):
            # smaller final chunks shorten the last-store tail
            chunks = [4] * 7 + [2] * 2
            g0 = 0
            for npc in chunks:
                yc = yp.tile([P, TPC, C], BF16, tag="yc")
                for t in range(0, npc, 2):
                    ps = psy.tile([P, 2, C], F32, tag="psy")
                    for tt in range(2):
                        g = g0 + t + tt
                        for k in range(KT):
                            nc.tensor.matmul(
                                ps[:, tt, :],
                                xTb[:, k, g * P:(g + 1) * P], g_sb[:, k, :],
                                start=(k == 0), stop=(k == KT - 1),
                            )
                    copy_out(yc[:, t:t + 2, :], ps[:])
                out_eng = {"sync": nc.sync, "scalar": nc.scalar,
                           "gpsimd": nc.gpsimd}[CFG.get("out_ring", "sync")]
                out_eng.dma_start(
                    out=out_d[g0 * P:(g0 + npc) * P, :].rearrange(
                        "(t p) c -> p t c", p=P
                    ),
                    in_=yc[:, 0:npc, :],
                )
                g0 += npc
    else:
        for ch in range(NCH):
            nc.scalar.dma_start(
                out=out_d[ch * C:(ch + 1) * C, :].rearrange(
                    "(t p) c -> p t c", p=P
                ),
                in_=g_sb[:],
            )


_NC_CACHE = None


def _get_nc():
    global _NC_CACHE
    if _NC_CACHE is None:
        _NC_CACHE = build_bass()
    return _NC_CACHE


def make_in_maps(x, w_qkv, w_proj, temperature):
    """Host-side prep: transpose/cast so the kernel only does plain
    contiguous DMA loads."""
    import ml_dtypes

    bf = ml_dtypes.bfloat16
    f8 = ml_dtypes.float8_e4m3
    x = np.asarray(x, dtype=np.float32)
    w_qkv = np.asarray(w_qkv, dtype=np.float32)
    w_proj = np.asarray(w_proj, dtype=np.float32)
    temperature = np.ascontiguousarray(np.asarray(temperature, dtype=np.float32))

    wq = w_qkv[0:C]          # [512 qch, 512 cin]
    wk = w_qkv[C:2 * C]
    wqkT_f = np.concatenate([wq.T, wk.T], axis=1)            # [cin, 1024]
    wqkT = np.ascontiguousarray(wqkT_f.astype(bf))
    wqk8 = np.ascontiguousarray(wqkT_f.astype(f8))
    wv = np.ascontiguousarray(w_qkv[2 * C:3 * C].astype(bf))  # [he, cin]
    wpT = np.ascontiguousarray(w_proj.T.astype(bf))           # [hd, cout]
    ident = np.eye(P, dtype=np.float32)

    def interleave_x8(xb):
        xa = xb.astype(f8).reshape(NTP, 2, P, KT, P)     # [tp, ab, k, i, m]
        rev = xa[:, :, :, :, ::-1]
        perm = rev.transpose(2, 0, 3, 4, 1)              # [k, tp, i, j, ab]
        return np.ascontiguousarray(perm.reshape(P, NT * C))

    drsw = CFG.get("xg_mode", "dr") == "drsw"
    maps = []
    for b in range(N_CORES):
        maps.append({
            "x8": interleave_x8(x[b]) if drsw
            else np.ascontiguousarray(x[b].astype(f8)),
            "xTb": np.ascontiguousarray(x[b].T.astype(bf)),
            "wqkT": wqkT,
            "wqk8": wqk8,
            "wv": wv,
            "wpT": wpT,
            "temperature": temperature,
            "ident": ident,
        })
    return maps


def kernel(**inputs) -> np.ndarray:
    from concourse.bass_utils import run_bass_kernel_spmd

    nc = _get_nc()
    in_maps = make_in_maps(
        inputs["x"], inputs["w_qkv"], inputs["w_proj"], inputs["temperature"]
    )
    res = run_bass_kernel_spmd(nc, in_maps, core_ids=list(range(N_CORES)))
    if CFG.get("c_transposed", False):
        return np.stack(
            [np.asarray(res.results[b]["out"], dtype=np.float32).T
             for b in range(N_CORES)],
            axis=0,
        )
    return np.stack(
        [np.asarray(res.results[b]["out"], dtype=np.float32)
         for b in range(N_CORES)],
        axis=0,
    )
